# revision 10
# baseline (speedup 1.0000x reference)
"""Trainium2 Bass kernel for nn_AlbertEmbeddings (scatter_memory).

Sharding: data-parallel over batch B=64 -> 8 rows per NeuronCore.
Each core: gathers word embeddings for its rows (f32, exact output path),
runs the sense-context pipeline (self-attn + textCNN + projections) in bf16
feature-major layout, scatters contexts back, LayerNorms, writes [8*128, 768].
Host does only index plumbing (nonzero positions, onehot/selection matrices).
"""
import sys

if "/opt/trn_rl_repo" not in sys.path:
    sys.path.insert(0, "/opt/trn_rl_repo")

import numpy as np
import ml_dtypes

import concourse.bass as bass
import concourse.bacc as bacc
import concourse.mybir as mybir
import concourse.tile as tile
from concourse.bass_utils import run_bass_kernel_spmd
from concourse.masks import make_identity

F32 = mybir.dt.float32
BF16 = mybir.dt.bfloat16
I32 = mybir.dt.int32
AF = mybir.ActivationFunctionType
ALU = mybir.AluOpType
BF = ml_dtypes.bfloat16

# ---- problem constants (hardcoded per spec) --------------------------------
B, L, E = 64, 128, 768
V = 30000
SEG = 64
NCORE = 8
NB = B // NCORE            # 8 rows per core
KSEG = 16                  # k-items per segment per core
KC = 2 * KSEG              # 32 k-items per core
NXS = 16                   # X sequences per core (8 per segment)
DSEQ = 132                 # de sequences per core, padded (128 real + 4 pad)
DTOK = DSEQ * 20           # 2640
DPAD = 2688                # padded to 21 * 128 for gather tiles
ET = E // 128              # 6 feature tiles
D5 = 5 * E                 # 3840
SCL = 1.0 / float(np.sqrt(np.float32(E)))
EPS = 1e-12
FILTERS = (1, 2, 3, 4, 5)

_CACHE = {}


def _build():
    nc = bacc.Bacc("TRN2", target_bir_lowering=False, debug=False,
                   num_devices=NCORE)

    def din(name, shape, dt=F32):
        return nc.declare_dram_parameter(name, list(shape), dt, isOutput=False)

    # --- DRAM inputs (per core) ---
    wg_idx = din("wg_idx", [NB * L, 1], I32)
    x_idx = din("x_idx", [NXS * SEG, 1], I32)
    d_idx = din("d_idx", [DPAD, 1], I32)
    Wword = din("Wword", [V, E], F32)
    pos_e = din("pos_e", [L, E], F32)
    Wtype = din("Wtype", [2, E], F32)
    toh = din("toh", [NB, 2, L], F32)
    sc = din("sc", [NB, KC, L], F32)
    sel = din("sel", [NXS, KC], F32)
    maskX = din("maskX", [128, 128], F32)
    maskD = din("maskD", [128, 128], F32)
    Wq = din("Wq", [E, E], BF16)
    Wk = din("Wk", [E, E], BF16)
    Wv = din("Wv", [E, E], BF16)
    attnB = din("attnB", [E, 3], F32)
    cnnS = [din(f"cnnS{f}", [f * ET, 128, E], BF16) for f in FILTERS]
    cnnT = [din(f"cnnT{f}", [f * ET, 128, E], BF16) for f in FILTERS]
    cnnSB = din("cnnSB", [128, 5 * ET], F32)
    cnnTB = din("cnnTB", [128, 5 * ET], F32)
    Wproj = din("Wproj", [D5, D5], BF16)
    WprojS = din("WprojS", [D5, D5], BF16)
    bproj = din("bproj", [1, D5], BF16)
    bprojS = din("bprojS", [1, D5], BF16)
    Wout = din("Wout", [E, E], BF16)
    bout = din("bout", [1, E], BF16)
    ln_g = din("ln_g", [128, E], F32)
    ln_b = din("ln_b", [128, E], F32)
    out = nc.declare_dram_parameter("out", [NB * L, E], F32, isOutput=True)

    with tile.TileContext(nc) as tc:
        consts = tc.alloc_tile_pool(name="consts", bufs=1)
        persist = tc.alloc_tile_pool(name="persist", bufs=1)

        # --- constants in SBUF ---
        idf = consts.tile([128, 128], F32)
        make_identity(nc, idf[:])
        idb = consts.tile([128, 128], BF16)
        nc.vector.tensor_copy(out=idb[:], in_=idf[:])
        ones_b = consts.tile([1, 128], BF16)
        nc.gpsimd.memset(ones_b[:], 1.0)
        mx_sb = consts.tile([128, 128], F32)
        nc.sync.dma_start(out=mx_sb[:], in_=maskX[:])
        md_sb = consts.tile([128, 128], F32)
        nc.sync.dma_start(out=md_sb[:], in_=maskD[:])
        sel_sb = consts.tile([NXS, KC], F32)
        nc.sync.dma_start(out=sel_sb[:], in_=sel[:])
        # attn biases: sbuf [128, ET*3], col = et*3 + {0:q,1:k,2:v}
        ab_sb = consts.tile([128, 3 * ET], F32)
        for et in range(ET):
            nc.sync.dma_start(out=ab_sb[:, et * 3:(et + 1) * 3],
                              in_=attnB[et * 128:(et + 1) * 128, :])
        csb_sb = consts.tile([128, 5 * ET], F32)
        nc.sync.dma_start(out=csb_sb[:], in_=cnnSB[:])
        ctb_sb = consts.tile([128, 5 * ET], F32)
        nc.sync.dma_start(out=ctb_sb[:], in_=cnnTB[:])

        def load_w66(param, pool=None):  # [E,E] -> 6 tiles [128, E]
            pool = pool or consts
            ts = []
            for et in range(ET):
                t = pool.tile([128, E], BF16, tag=f"w66_{param.name}_{et}", name=f"w66_{param.name}_{et}")
                nc.sync.dma_start(out=t[:], in_=param[et * 128:(et + 1) * 128, :])
                ts.append(t)
            return ts

        wq_t = load_w66(Wq)
        wk_t = load_w66(Wk)
        wv_t = load_w66(Wv)

        def gather128(pool, idx_param, off, tag="gath"):
            ix = pool.tile([128, 1], I32, tag="gix")
            nc.sync.dma_start(out=ix[:], in_=idx_param[off:off + 128, :])
            g = pool.tile([128, E], F32, tag=tag)
            nc.gpsimd.indirect_dma_start(
                out=g[:], out_offset=None, in_=Wword[:],
                in_offset=bass.IndirectOffsetOnAxis(ap=ix[:, :1], axis=0))
            return g

        def transpose_in(pool, pp, g, dst_tiles, col0):
            for et in range(ET):
                tp = pp.tile([128, 128], F32, tag="tr_ps")
                nc.tensor.transpose(out=tp[:], in_=g[:, et * 128:(et + 1) * 128],
                                    identity=idf[:])
                nc.vector.tensor_copy(out=dst_tiles[et][:, col0:col0 + 128],
                                      in_=tp[:])

        # feature-major linear: dst[et][:, :ncols] = W.T @ src (+ bias/partition)
        def linear_fm(dst, src, w_t, bias_ch, ncols, sp, pp):
            nch = [512] * (ncols // 512) + ([ncols % 512] if ncols % 512 else [])
            for eo in range(ET):
                c0 = 0
                for cn in nch:
                    ps = pp.tile([128, 512], F32, tag="mm512")
                    for ei in range(ET):
                        nc.tensor.matmul(
                            out=ps[:, :cn],
                            lhsT=w_t[ei][:, eo * 128:(eo + 1) * 128],
                            rhs=src[ei][:, c0:c0 + cn],
                            start=(ei == 0), stop=(ei == ET - 1))
                    nc.vector.tensor_scalar_add(
                        out=dst[eo][:, c0:c0 + cn], in0=ps[:, :cn],
                        scalar1=ab_sb[:, eo * 3 + bias_ch:eo * 3 + bias_ch + 1])
                    c0 += cn

        # token-major V block: rows = nt tokens at col0, cols = E
        def v_block(src, w_t, col0, nt, sp, pp):
            vt = sp.tile([128, E], BF16, tag="v_sb")
            for n0, cn in ((0, 512), (512, 256)):
                ps = pp.tile([128, 512], F32, tag="mm512")
                for ei in range(ET):
                    nc.tensor.matmul(
                        out=ps[:nt, :cn],
                        lhsT=src[ei][:, col0:col0 + nt],
                        rhs=w_t[ei][:, n0:n0 + cn],
                        start=(ei == 0), stop=(ei == ET - 1))
                nc.vector.tensor_copy(out=vt[:nt, n0:n0 + cn], in_=ps[:nt, :cn])
            return vt

        # packed-group attention: scores -> softmax -> A.T -> attn_out (+bv)
        def attn_group(qt, kt, vt, mask_sb, col0, np_, dst, sp, pp):
            ps = pp.tile([128, 128], F32, tag="sc_ps")
            for ei in range(ET):
                nc.tensor.matmul(out=ps[:np_, :np_],
                                 lhsT=qt[ei][:, col0:col0 + np_],
                                 rhs=kt[ei][:, col0:col0 + np_],
                                 start=(ei == 0), stop=(ei == ET - 1))
            s_sb = sp.tile([128, 128], F32, tag="s_sb")
            nc.vector.tensor_tensor(out=s_sb[:np_, :np_], in0=ps[:np_, :np_],
                                    in1=mask_sb[:np_, :np_], op=ALU.add)
            mx = sp.tile([128, 1], F32, tag="mx")
            nc.vector.reduce_max(out=mx[:np_], in_=s_sb[:np_, :np_],
                                 axis=mybir.AxisListType.X)
            nmx = sp.tile([128, 1], F32, tag="nmx")
            nc.scalar.mul(out=nmx[:np_], in_=mx[:np_], mul=-SCL)
            ex = sp.tile([128, 128], BF16, tag="ex")
            nc.scalar.activation(out=ex[:np_, :np_], in_=s_sb[:np_, :np_],
                                 func=AF.Exp, bias=nmx[:np_, :1], scale=SCL)
            sm = sp.tile([128, 1], F32, tag="sm")
            nc.vector.reduce_sum(out=sm[:np_], in_=ex[:np_, :np_],
                                 axis=mybir.AxisListType.X)
            rs = sp.tile([128, 1], F32, tag="rs")
            nc.vector.reciprocal(out=rs[:np_], in_=sm[:np_])
            an = sp.tile([128, 128], BF16, tag="an")
            nc.vector.tensor_scalar_mul(out=an[:np_, :np_], in0=ex[:np_, :np_],
                                        scalar1=rs[:np_, :1])
            tp = pp.tile([128, 128], BF16, tag="at_ps")
            nc.tensor.transpose(out=tp[:np_, :np_], in_=an[:np_, :np_],
                                identity=idb[:np_, :np_])
            at = sp.tile([128, 128], BF16, tag="at_sb")
            nc.vector.tensor_copy(out=at[:np_, :np_], in_=tp[:np_, :np_])
            for eo in range(ET):
                ps2 = pp.tile([128, 128], F32, tag="ao_ps")
                nc.tensor.matmul(out=ps2[:, :np_],
                                 lhsT=vt[:np_, eo * 128:(eo + 1) * 128],
                                 rhs=at[:np_, :np_], start=True, stop=True)
                nc.vector.tensor_scalar_add(
                    out=dst[eo][:, col0:col0 + np_], in0=ps2[:, :np_],
                    scalar1=ab_sb[:, eo * 3 + 2:eo * 3 + 3])

        # textCNN: conv(f=1..5) + relu + maxpool -> featsT row-tiles [128, nseq]
        def conv_block(src_fm, w_params, bias_sb, feats, nseq, slen, chunks, pp):
            with tc.tile_pool(name="cw", bufs=2) as wp, \
                 tc.tile_pool(name="cs", bufs=3) as sp:
                for fi, f in enumerate(FILTERS):
                    wt = wp.tile([128, f * ET * E], BF16, tag="convw")
                    nc.sync.dma_start(
                        out=wt[:].rearrange("p (a c) -> p a c", a=f * ET),
                        in_=w_params[fi][:].rearrange("a p c -> p a c"))
                    nh = slen - f + 1
                    for ct in range(ET):
                        for c0, ns in chunks:
                            ps = pp.tile([128, 512], F32, tag="cv_ps")
                            pv = ps[:, :ns * nh].rearrange("p (n h) -> p n h", n=ns)
                            for j in range(f):
                                for ei in range(ET):
                                    rv = src_fm[ei][:, :nseq * slen].rearrange(
                                        "p (n h) -> p n h", h=slen)
                                    nc.tensor.matmul(
                                        out=pv[:, :, :],
                                        lhsT=wt[:, ((j * ET + ei) * E + ct * 128):
                                                ((j * ET + ei) * E + ct * 128 + 128)],
                                        rhs=rv[:, c0:c0 + ns, j:j + nh],
                                        start=(j == 0 and ei == 0),
                                        stop=(j == f - 1 and ei == ET - 1))
                            red = sp.tile([128, 32], F32, tag="red")
                            nc.vector.reduce_max(out=red[:, :ns], in_=pv[:, :, :],
                                                 axis=mybir.AxisListType.X)
                            nc.scalar.activation(
                                out=feats[fi * ET + ct][:, c0:c0 + ns],
                                in_=red[:, :ns], func=AF.Relu,
                                bias=bias_sb[:, fi * ET + ct:fi * ET + ct + 1],
                                scale=1.0)

        # ======== X path (scoped: XT, XA) ==================================
        poolF = [persist.tile([128, NXS], BF16, tag=f"pf{i}", name=f"pf{i}") for i in range(5 * ET)]
        with tc.tile_pool(name="xpath", bufs=1) as xp:
            XT = [xp.tile([128, NXS * SEG], BF16, tag=f"XT{e}", name=f"XT{e}") for e in range(ET)]
            XA = [xp.tile([128, NXS * SEG], BF16, tag=f"XA{e}", name=f"XA{e}") for e in range(ET)]
            with (
                tc.tile_pool(name="xg", bufs=3) as gp,
                tc.tile_pool(name="xg_ps", bufs=4, space="PSUM") as gps,
            ):
                for i in range(NXS * SEG // 128):
                    g = gather128(gp, x_idx, i * 128)
                    transpose_in(gp, gps, g, XT, i * 128)
            with (
                tc.tile_pool(name="xat_sb", bufs=3) as sp,
                tc.tile_pool(name="xat_q", bufs=1) as qp,
                tc.tile_pool(name="xat_ps", bufs=2, space="PSUM") as pp,
            ):
                QTx = [qp.tile([128, NXS * SEG], BF16, tag=f"qtx{e}", name=f"qtx{e}")
                       for e in range(ET)]
                KTx = [qp.tile([128, NXS * SEG], BF16, tag=f"ktx{e}", name=f"ktx{e}")
                       for e in range(ET)]
                linear_fm(QTx, XT, wq_t, 0, NXS * SEG, sp, pp)
                linear_fm(KTx, XT, wk_t, 1, NXS * SEG, sp, pp)
                for pk in range(NXS * SEG // 128):
                    vt = v_block(XT, wv_t, pk * 128, 128, sp, pp)
                    attn_group(QTx, KTx, vt, mx_sb, pk * 128, 128, XA, sp, pp)
            with tc.tile_pool(name="xc_ps", bufs=4, space="PSUM") as pp:
                conv_block(XA, cnnS, csb_sb, poolF, NXS, SEG,
                           [(0, 8), (8, 8)], pp)

        # ======== de path ==================================================
        tagF = [persist.tile([128, DSEQ], BF16, tag=f"tf{i}", name=f"tf{i}") for i in range(5 * ET)]
        with tc.tile_pool(name="dpath", bufs=1) as dp:
            DA = [dp.tile([128, DTOK], BF16, tag=f"DA{e}", name=f"DA{e}") for e in range(ET)]
            with tc.tile_pool(name="depool", bufs=1) as dep:
                deT = [dep.tile([128, DPAD], BF16, tag=f"deT{e}", name=f"deT{e}")
                       for e in range(ET)]
                with (
                    tc.tile_pool(name="dg", bufs=3) as gp,
                    tc.tile_pool(name="dg_ps", bufs=4, space="PSUM") as gps,
                ):
                    for i in range(DPAD // 128):
                        g = gather128(gp, d_idx, i * 128)
                        transpose_in(gp, gps, g, deT, i * 128)
                with (
                    tc.tile_pool(name="dat_sb", bufs=3) as sp,
                    tc.tile_pool(name="dat_q", bufs=1) as qp,
                    tc.tile_pool(name="dat_ps", bufs=2, space="PSUM") as pp,
                ):
                    QTd = [qp.tile([128, DTOK], BF16, tag=f"qtd{e}", name=f"qtd{e}")
                           for e in range(ET)]
                    KTd = [qp.tile([128, DTOK], BF16, tag=f"ktd{e}", name=f"ktd{e}")
                           for e in range(ET)]
                    linear_fm(QTd, deT, wq_t, 0, DTOK, sp, pp)
                    linear_fm(KTd, deT, wk_t, 1, DTOK, sp, pp)
                    for g_ in range(DTOK // 120):
                        vt = v_block(deT, wv_t, g_ * 120, 120, sp, pp)
                        attn_group(QTd, KTd, vt, md_sb, g_ * 120, 120, DA, sp, pp)
            with tc.tile_pool(name="dc_ps", bufs=4, space="PSUM") as pp:
                conv_block(DA, cnnT, ctb_sb, tagF, DSEQ, 20,
                           [(i * 25, 25) for i in range(5)] + [(125, 7)], pp)

        # ======== projections + sense attention + ctx ======================
        ctx_sb = persist.tile([KC, E], F32, tag="ctx")
        with (
            tc.tile_pool(name="pj_w", bufs=2) as wp,
            tc.tile_pool(name="pj_sb", bufs=1) as sp,
            tc.tile_pool(name="pj_c", bufs=1) as cp,
            tc.tile_pool(name="pj_ps", bufs=1, space="PSUM") as pp,
        ):
            NCH = 8  # 8 psum chunks of 480, tag_tot split per-t (base partition 0)
            wo_t = load_w66(Wout, cp)
            bprj_sb = cp.tile([1, D5], BF16)
            nc.sync.dma_start(out=bprj_sb[:], in_=bproj[:])
            bprs_sb = cp.tile([1, D5], BF16)
            nc.sync.dma_start(out=bprs_sb[:], in_=bprojS[:])
            bout_sb = cp.tile([1, E], BF16)
            nc.sync.dma_start(out=bout_sb[:], in_=bout[:])
            tag_t = [sp.tile([KC, D5], BF16, tag=f"tag_t{t}", name=f"tag_t{t}")
                     for t in range(4)]
            pool_sb = sp.tile([NXS, D5], F32, tag="pool_s")
            for feats, m_, wparam, bias_sb_ in (
                (tagF, 128, Wproj, bprj_sb),
                (poolF, NXS, WprojS, bprs_sb),
            ):
                pss = [pp.tile([128, 480], F32, tag=f"prj{i}", name=f"prj{i}") for i in range(NCH)]
                for i in range(NCH):
                    nc.tensor.matmul(out=pss[i][:m_], lhsT=ones_b[:1, :m_],
                                     rhs=bias_sb_[:1, i * 480:(i + 1) * 480],
                                     start=True, stop=False)
                for kt in range(D5 // 128):
                    wr = wp.tile([128, D5], BF16, tag="wrow")
                    nc.sync.dma_start(out=wr[:],
                                      in_=wparam[kt * 128:(kt + 1) * 128, :])
                    for i in range(NCH):
                        nc.tensor.matmul(out=pss[i][:m_],
                                         lhsT=feats[kt][:, :m_],
                                         rhs=wr[:, i * 480:(i + 1) * 480],
                                         start=False, stop=(kt == D5 // 128 - 1))
                for i in range(NCH):
                    if m_ == 128:
                        for t in range(4):
                            nc.vector.tensor_copy(
                                out=tag_t[t][:, i * 480:(i + 1) * 480],
                                in_=pss[i][t * KC:(t + 1) * KC, :])
                    else:
                        nc.vector.tensor_copy(
                            out=pool_sb[:, i * 480:(i + 1) * 480],
                            in_=pss[i][:m_])
            # replicate pool_s rows to k-items: rep = sel.T @ pool_s  [32,3840]
            rep_sb = sp.tile([KC, D5], F32, tag="accrep")
            for i in range(NCH):
                ps3 = pp.tile([KC, 480], F32, tag="prj0")
                nc.tensor.matmul(out=ps3[:], lhsT=sel_sb[:, :],
                                 rhs=pool_sb[:, i * 480:(i + 1) * 480],
                                 start=True, stop=True)
                nc.vector.tensor_copy(out=rep_sb[:, i * 480:(i + 1) * 480],
                                      in_=ps3[:])
            # scores[k, t] = rep . tag_tot[t*32+k, :]
            scr = sp.tile([KC, 4], F32, tag="scr")
            junk = sp.tile([KC, D5], F32, tag="tmp5", name="junk")
            for t in range(4):
                nc.vector.tensor_tensor(
                    out=junk[:], in0=rep_sb[:], in1=tag_t[t][:], op=ALU.mult)
                nc.vector.reduce_sum(out=scr[:, t:t + 1], in_=junk[:],
                                     axis=mybir.AxisListType.X)
            smx = sp.tile([KC, 1], F32, tag="smx")
            nc.vector.reduce_max(out=smx[:], in_=scr[:], axis=mybir.AxisListType.X)
            nsm = sp.tile([KC, 1], F32, tag="nsm")
            nc.scalar.mul(out=nsm[:], in_=smx[:], mul=-1.0)
            ex4 = sp.tile([KC, 4], F32, tag="ex4")
            nc.scalar.activation(out=ex4[:], in_=scr[:], func=AF.Exp,
                                 bias=nsm[:, :1], scale=1.0)
            ss4 = sp.tile([KC, 1], F32, tag="ss4")
            nc.vector.reduce_sum(out=ss4[:], in_=ex4[:], axis=mybir.AxisListType.X)
            rc4 = sp.tile([KC, 1], F32, tag="rc4")
            nc.vector.reciprocal(out=rc4[:], in_=ss4[:])
            att = sp.tile([KC, 4], F32, tag="att")
            nc.vector.tensor_scalar_mul(out=att[:], in0=ex4[:], scalar1=rc4[:, :1])
            # ctx5 = sum_t att[:,t] * tag_tot[t-block]   [32, 3840]
            acc = sp.tile([KC, D5], F32, tag="accrep", name="acc")
            tmp = sp.tile([KC, D5], F32, tag="tmp5", name="tmp")
            nc.vector.tensor_scalar_mul(out=acc[:], in0=tag_t[0][:],
                                        scalar1=att[:, 0:1])
            for t in range(1, 4):
                nc.vector.tensor_scalar_mul(out=tmp[:], in0=tag_t[t][:],
                                            scalar1=att[:, t:t + 1])
                nc.vector.tensor_tensor(out=acc[:], in0=acc[:], in1=tmp[:],
                                        op=ALU.add)
            # mean over 5 chunks of 768 -> [32, 768] bf16
            c5 = sp.tile([KC, E], F32, tag="c5")
            nc.vector.tensor_tensor(out=c5[:], in0=acc[:, 0:E], in1=acc[:, E:2 * E],
                                    op=ALU.add)
            for i in range(2, 5):
                nc.vector.tensor_tensor(out=c5[:], in0=c5[:],
                                        in1=acc[:, i * E:(i + 1) * E], op=ALU.add)
            c5b = sp.tile([KC, E], BF16, tag="c5b")
            nc.scalar.mul(out=c5b[:], in_=c5[:], mul=0.2)
            ctxT = []
            for et in range(ET):
                tp = pp.tile([128, KC], BF16, tag="prj1")
                nc.tensor.transpose(out=tp[:], in_=c5b[:, et * 128:(et + 1) * 128],
                                    identity=idb[:KC, :KC])
                t_sb = sp.tile([128, KC], BF16, tag=f"ctxT{et}")
                nc.vector.tensor_copy(out=t_sb[:], in_=tp[:])
                ctxT.append(t_sb)
            # ctx = ctx5_mean @ Wout + bout  [32, 768] f32 token-major
            for n0, cn in ((0, 512), (512, 256)):
                ps4 = pp.tile([KC, 512], F32, tag="prj2")
                nc.tensor.matmul(out=ps4[:, :cn], lhsT=ones_b[:1, :KC],
                                 rhs=bout_sb[:1, n0:n0 + cn], start=True, stop=False)
                for ei in range(ET):
                    nc.tensor.matmul(out=ps4[:, :cn], lhsT=ctxT[ei][:, :],
                                     rhs=wo_t[ei][:, n0:n0 + cn],
                                     start=False, stop=(ei == ET - 1))
                nc.vector.tensor_copy(out=ctx_sb[:, n0:n0 + cn], in_=ps4[:, :cn])

        # ======== assemble embeddings, LayerNorm, write out ================
        with (
            tc.tile_pool(name="o_g", bufs=3) as gp,
            tc.tile_pool(name="o_sb", bufs=3) as sp,
            tc.tile_pool(name="o_c", bufs=1) as cp,
            tc.tile_pool(name="o_ps", bufs=4, space="PSUM") as pp,
        ):
            pos_sb = cp.tile([128, E], F32)
            nc.sync.dma_start(out=pos_sb[:], in_=pos_e[:])
            wty_sb = cp.tile([2, E], F32)
            nc.sync.dma_start(out=wty_sb[:], in_=Wtype[:])
            lng_sb = cp.tile([128, E], F32)
            nc.sync.dma_start(out=lng_sb[:], in_=ln_g[:])
            lnb_sb = cp.tile([128, E], F32)
            nc.sync.dma_start(out=lnb_sb[:], in_=ln_b[:])
            eps_sb = cp.tile([128, 1], F32)
            nc.gpsimd.memset(eps_sb[:], EPS)
            toh_sb = cp.tile([2, NB * L], F32)
            sc_sb = cp.tile([KC, NB * L], F32)
            for r in range(NB):
                nc.sync.dma_start(out=toh_sb[:, r * L:(r + 1) * L], in_=toh[r])
                nc.sync.dma_start(out=sc_sb[:, r * L:(r + 1) * L], in_=sc[r])
            for r in range(NB):
                wt_ = gather128(gp, wg_idx, r * 128)
                emb = sp.tile([128, E], F32, tag="emb")
                nc.vector.tensor_tensor(out=emb[:], in0=wt_[:],
                                        in1=pos_sb[:], op=ALU.add)
                for n0, cn in ((0, 512), (512, 256)):
                    ps = pp.tile([128, 512], F32, tag="o_ps")
                    nc.tensor.matmul(out=ps[:, :cn],
                                     lhsT=toh_sb[:, r * 128:(r + 1) * 128],
                                     rhs=wty_sb[:, n0:n0 + cn],
                                     start=True, stop=False)
                    nc.tensor.matmul(out=ps[:, :cn],
                                     lhsT=sc_sb[:, r * 128:(r + 1) * 128],
                                     rhs=ctx_sb[:, n0:n0 + cn],
                                     start=False, stop=True)
                    nc.vector.tensor_tensor(out=emb[:, n0:n0 + cn],
                                            in0=emb[:, n0:n0 + cn],
                                            in1=ps[:, :cn], op=ALU.add)
                s1 = sp.tile([128, 1], F32, tag="s1")
                nc.vector.reduce_sum(out=s1[:], in_=emb[:],
                                     axis=mybir.AxisListType.X)
                nmu = sp.tile([128, 1], F32, tag="nmu")
                nc.scalar.mul(out=nmu[:], in_=s1[:], mul=-1.0 / E)
                cent = sp.tile([128, E], F32, tag="cent")
                nc.vector.tensor_scalar_add(out=cent[:], in0=emb[:],
                                            scalar1=nmu[:, :1])
                sq = sp.tile([128, E], F32, tag="sq")
                vs = sp.tile([128, 1], F32, tag="vs")
                nc.scalar.activation(out=sq[:], in_=cent[:], func=AF.Square,
                                     accum_out=vs[:])
                sd = sp.tile([128, 1], F32, tag="sd")
                nc.scalar.activation(out=sd[:], in_=vs[:], func=AF.Sqrt,
                                     bias=eps_sb[:, :1], scale=1.0 / E)
                rstd = sp.tile([128, 1], F32, tag="rstd")
                nc.vector.reciprocal(out=rstd[:], in_=sd[:])
                nrm = sp.tile([128, E], F32, tag="nrm")
                nc.vector.tensor_scalar_mul(out=nrm[:], in0=cent[:],
                                            scalar1=rstd[:, :1])
                nc.vector.tensor_tensor(out=nrm[:], in0=nrm[:], in1=lng_sb[:],
                                        op=ALU.mult)
                nc.vector.tensor_tensor(out=nrm[:], in0=nrm[:], in1=lnb_sb[:],
                                        op=ALU.add)
                nc.sync.dma_start(out=out[r * 128:(r + 1) * 128, :], in_=nrm[:])

        persist.release()
        consts.release()

    nc.compile()
    return nc


def _host_prep(input_ids, token_type_ids, attention_mask, input_tag_ids,
               input_def_ids, input_tag_label, params):
    """Build the 8 per-core input maps. Host work = index plumbing only."""
    ids = np.asarray(input_ids, dtype=np.int64).astype(np.int32)
    tt = np.asarray(token_type_ids, dtype=np.int64).astype(np.int32)
    am = np.asarray(attention_mask, dtype=np.int64).astype(np.int32)
    dfi = np.asarray(input_def_ids, dtype=np.int64).astype(np.int32)
    tl = np.asarray(input_tag_label, dtype=np.int64).astype(np.int32)

    pf = tt + am
    K = 2 * B
    kb, kp = [], []
    for sv in (1, 2):
        lab = np.where(pf == sv, tl, 5)
        b_i, p_i = np.nonzero(lab != 5)
        b_i, p_i = b_i[:K], p_i[:K]
        if len(b_i) < K:  # replicate jnp.nonzero(size=K) zero padding
            pad = K - len(b_i)
            b_i = np.concatenate([b_i, np.zeros(pad, np.int64)])
            p_i = np.concatenate([p_i, np.zeros(pad, np.int64)])
        kb.append(b_i.astype(np.int32))
        kp.append(p_i.astype(np.int32))

    f32 = np.float32
    p = params
    shared = {
        "Wword": np.asarray(p["W_word"], f32),
        "pos_e": np.asarray(p["W_pos"][:L], f32),
        "Wtype": np.asarray(p["W_type"], f32),
        "Wq": np.asarray(p["attn_Wq"], f32).astype(BF),
        "Wk": np.asarray(p["attn_Wk"], f32).astype(BF),
        "Wv": np.asarray(p["attn_Wv"], f32).astype(BF),
        "attnB": np.ascontiguousarray(
            np.stack([np.asarray(p["attn_bq"], f32),
                      np.asarray(p["attn_bk"], f32),
                      np.asarray(p["attn_bv"], f32)], axis=1)),
        "Wproj": np.asarray(p["cnn_proj_w"], f32).astype(BF),
        "WprojS": np.asarray(p["cnn_s_proj_w"], f32).astype(BF),
        "bproj": np.asarray(p["cnn_proj_b"], f32).astype(BF)[None, :],
        "bprojS": np.asarray(p["cnn_s_proj_b"], f32).astype(BF)[None, :],
        "Wout": np.asarray(p["proj_w"], f32).astype(BF),
        "bout": np.asarray(p["proj_b"], f32).astype(BF)[None, :],
        "ln_g": np.ascontiguousarray(
            np.broadcast_to(np.asarray(p["ln_g"], f32), (128, E))),
        "ln_b": np.ascontiguousarray(
            np.broadcast_to(np.asarray(p["ln_b"], f32), (128, E))),
    }
    # conv weights: w[c_out, 1, f, e_in] -> [f*ET, 128, E]; block (j, ei) is
    # the lhsT [e_in 128, c_out E] slab for shift j, e_in tile ei.
    for nm, key in (("cnnS", "cnn_s_w"), ("cnnT", "cnn_tag_w")):
        for fi, f in enumerate(FILTERS):
            w = np.asarray(p[key][fi], f32)  # [E_out, 1, f, E_in]
            m = w[:, 0, :, :].transpose(1, 2, 0)  # [f, e_in, c_out]
            shared[f"{nm}{f}"] = np.ascontiguousarray(
                m.reshape(f * ET, 128, E)).astype(BF)
    for nm, key in (("cnnSB", "cnn_s_b"), ("cnnTB", "cnn_tag_b")):
        bcols = np.zeros((128, 5 * ET), f32)
        for fi in range(5):
            bf_ = np.asarray(p[key][fi], f32)
            for ct in range(ET):
                bcols[:, fi * ET + ct] = bf_[ct * 128:(ct + 1) * 128]
        shared[nm] = bcols
    mX = np.full((128, 128), -1e9, f32)
    for i in range(2):
        mX[i * 64:(i + 1) * 64, i * 64:(i + 1) * 64] = 0.0
    mD = np.full((128, 128), -1e9, f32)
    for i in range(6):
        mD[i * 20:(i + 1) * 20, i * 20:(i + 1) * 20] = 0.0
    shared["maskX"] = mX
    shared["maskD"] = mD

    in_maps = []
    for c in range(NCORE):
        rows = np.arange(c * NB, (c + 1) * NB)
        m = dict(shared)
        m["wg_idx"] = np.ascontiguousarray(ids[rows].reshape(-1, 1))
        krow = np.concatenate([kb[s][c * KSEG:(c + 1) * KSEG] for s in (0, 1)])
        kpos = np.concatenate([kp[s][c * KSEG:(c + 1) * KSEG] for s in (0, 1)])
        kseg = np.concatenate([np.full(KSEG, s, np.int32) for s in (0, 1)])
        assert np.all((krow >= rows[0]) & (krow <= rows[-1])), \
            "k-item scattered outside its core's rows"
        x_ids = np.zeros((NXS, SEG), np.int32)
        seq_of_k = np.zeros(KC, np.int32)
        for s in (0, 1):
            sel_k = np.where(kseg == s)[0]
            uniq = []
            for j in sel_k:
                r = int(krow[j])
                if r not in uniq:
                    uniq.append(r)
                assert len(uniq) <= NXS // 2, "more than 8 unique rows/segment"
                seq_of_k[j] = s * (NXS // 2) + uniq.index(r)
            for u, r in enumerate(uniq):
                x_ids[s * (NXS // 2) + u] = ids[r, s * SEG:(s + 1) * SEG]
        m["x_idx"] = np.ascontiguousarray(x_ids.reshape(-1, 1))
        d_ids = np.zeros((DPAD,), np.int32)
        de = dfi[krow, kpos]  # [KC, 4, 20]
        d_ids[:4 * KC * 20] = de.transpose(1, 0, 2).reshape(-1)
        m["d_idx"] = np.ascontiguousarray(d_ids.reshape(-1, 1))
        selm = np.zeros((NXS, KC), f32)
        selm[seq_of_k, np.arange(KC)] = 1.0
        m["sel"] = selm
        tohm = np.zeros((NB, 2, L), f32)
        for r in range(NB):
            tohm[r, tt[rows[r]], np.arange(L)] = 1.0
        m["toh"] = tohm
        scm = np.zeros((NB, KC, L), f32)
        for j in range(KC):
            scm[krow[j] - rows[0], j, kpos[j]] = 1.0
        m["sc"] = scm
        in_maps.append(m)
    return in_maps


def kernel(**inputs):
    if "nc" not in _CACHE:
        _CACHE["nc"] = _build()
    nc = _CACHE["nc"]
    in_maps = _host_prep(**inputs)
    res = run_bass_kernel_spmd(nc, in_maps, list(range(NCORE)))
    outs = [res.results[i]["out"].reshape(NB, L, E) for i in range(NCORE)]
    return np.concatenate(outs, axis=0).astype(np.float32)
